# revision 24
# baseline (speedup 1.0000x reference)
"""Distributed causal multi-head attention for Trainium2 (8 NeuronCores).

Problem: B=2, S=2048, d_model=1024, 16 heads x 64 dims, causal softmax attention.

Strategy (tensor-parallel over heads + all-to-all for output projection):
  - Each core owns 2 heads (128 of the 1024 QKV features).
  - Host pre-transposes x -> X^T [1024, 4096] and casts inputs to bf16, so all
    on-chip matmuls consume feature-on-partition ("transposed") activations
    directly with no on-chip transposes of x.
  - Per core: Q^T/K^T/V^T = W^T-shard @ X^T (+bias), attention per (batch, head)
    in S^T layout ([k-partitions, q-free]) with exp (no max subtraction; scores
    are O(1) so fp32 exp is safe), causal masking via a single 128x128 upper-
    triangular mask on diagonal tiles, and denominators via an appended ones
    column on V (PE computes the partition-dim sums for free).
  - Both heads of a k-tile share one [128,1024] PSUM tile (adjacent banks) so a
    single ScalarE exp covers them; heads' S^T matmuls pack into the PE array
    via disjoint 64-row groups.
  - Normalization pre-collective (per-head denominators, fast-approx
    reciprocal), then one AllToAll per batch redistributes O^T from head-sharded
    to row-sharded; each core then computes its 2x256 output rows with full Wo.
  - Output f32; host reassembles the full [2, 2048, 1024].
"""
import os
import sys

sys.path.insert(0, "/opt/trn_rl_repo")

import numpy as np
import ml_dtypes

from concourse import bacc, mybir, tile
from concourse.bass_utils import run_bass_kernel_spmd

BF16 = mybir.dt.bfloat16
F32 = mybir.dt.float32

B, S, DM = 2, 2048, 1024
H, DK = 16, 64
N_CORES = 8
FPC = 128           # features per core = 2 heads x 64
RPC = S // N_CORES  # output rows per core per batch = 256
NKT = S // 128      # k-tiles per batch = 16
NQC = S // 512      # q-chunks per batch = 4
SCALE = 1.0 / 8.0   # 1/sqrt(64)

_cache = {}


def _build():
    nc = bacc.Bacc("TRN2", target_bir_lowering=False, debug=False, num_devices=N_CORES)

    xt = nc.dram_tensor("xt", [DM, B * S], BF16, kind="ExternalInput")
    # wpk[p, :]: [wq|wk|wv tiles (3*8*128)] + [wo (1024)] + [mask|ident (256)]
    wpk = nc.dram_tensor("wpk", [128, 3 * 8 * 128 + DM + 256], BF16, kind="ExternalInput")
    bpk = nc.dram_tensor("bpk", [FPC, 3], F32, kind="ExternalInput")
    out_ext = nc.dram_tensor("out", [B, S, DM], F32, kind="ExternalOutput")

    EXP = mybir.ActivationFunctionType.Exp
    IDENT = mybir.ActivationFunctionType.Identity

    with tile.TileContext(nc) as tc:
        with (
            tc.tile_pool(name="xtp", bufs=1) as xtp,
            tc.tile_pool(name="wts", bufs=1) as wts,
            tc.tile_pool(name="qkv", bufs=1) as qkvp,
            tc.tile_pool(name="vnat", bufs=1) as vnatp,
            tc.tile_pool(name="work", bufs=3) as work,
            tc.tile_pool(name="stage", bufs=2) as stagep,
            tc.tile_pool(name="outp", bufs=2) as outp,
            tc.tile_pool(name="psmm", bufs=2, space="PSUM") as psmm,
            tc.tile_pool(name="psS", bufs=2, space="PSUM") as psS,
            tc.tile_pool(name="psO", bufs=1, space="PSUM") as psO,
        ):
            # ---------- load packed weights/constants (2 DMAs) ----------
            WPK_N = 3 * 8 * 128 + DM + 256
            wpk_sb = wts.tile([128, WPK_N], BF16, tag="wpk", name="wpk_sb")
            nc.sync.dma_start(wpk_sb[:], wpk[:])
            bpk_sb = wts.tile([FPC, 3], F32, tag="bpk", name="bpk_sb")
            nc.sync.dma_start(bpk_sb[:], bpk[:])

            def wslice(pr, kc):
                o = (pr * 8 + kc) * 128
                return wpk_sb[:, o:o + 128]

            wq_sb = [wslice(0, kc) for kc in range(8)]
            wk_sb = [wslice(1, kc) for kc in range(8)]
            wv_sb = [wslice(2, kc) for kc in range(8)]
            wo_sb = wpk_sb[:, 3072:3072 + DM]
            mask_sb = wpk_sb[:, 3072 + DM:3072 + DM + 128]
            ident_sb = wpk_sb[:, 3072 + DM + 128:3072 + DM + 256]
            b_sb = {"q": bpk_sb[:, 0:1], "k": bpk_sb[:, 1:2], "v": bpk_sb[:, 2:3]}

            # xt loaded in column pieces, rc-major, so the first projection
            # chains unblock as soon as the first pieces land
            xt_sb = []
            for kc in range(8):
                t = xtp.tile([128, B * S], BF16, tag=f"xt{kc}", name=f"xt{kc}")
                xt_sb.append(t)
            for rp in range(4):
                for kc in range(8):
                    nc.sync.dma_start(
                        xt_sb[kc][:, rp * 1024:(rp + 1) * 1024],
                        xt[kc * 128:(kc + 1) * 128, rp * 1024:(rp + 1) * 1024],
                    )

            # ---------- phases 1-3 interleaved ----------
            # row-chunk rc feeds q-chunk (b, qi): projections for rc, then V
            # transposes for that q range, then attention for (b, qi). This
            # overlaps ScalarE exp work with TensorE projection matmuls.
            proj_sb = {}
            for name in ("q", "k", "v"):
                proj_sb[name] = qkvp.tile(
                    [128, B * S], BF16, tag=f"{name}T", name=f"{name}T"
                )
            qT, kT, vT = proj_sb["q"], proj_sb["k"], proj_sb["v"]
            w_by_name = {"q": wq_sb, "k": wk_sb, "v": wv_sb}
            v_nat = [[None] * NKT for _ in range(B)]
            ot_tiles = []

            for rc in range(8):
                b, qc = (0, rc) if rc < 4 else (1, rc - 4)
                # projections for this row chunk
                for name in ("q", "k", "v"):
                    ps = psmm.tile([128, 512], F32, tag="mm", name=f"ps_{name}{rc}")
                    for kc in range(8):
                        nc.tensor.matmul(
                            ps[:], w_by_name[name][kc],
                            xt_sb[kc][:, rc * 512:(rc + 1) * 512],
                            start=(kc == 0), stop=(kc == 7),
                        )
                    nc.scalar.activation(
                        proj_sb[name][:, rc * 512:(rc + 1) * 512], ps[:], IDENT,
                        bias=b_sb[name],
                    )
                # V natural (+ones cols) for this q range
                for kt in range(4 * qc, 4 * qc + 4):
                    ps = psmm.tile([128, 128], BF16, tag="mm", name=f"pst{b}_{kt}")
                    nc.tensor.transpose(
                        ps[:], vT[:, b * S + kt * 128: b * S + (kt + 1) * 128],
                        ident_sb,
                    )
                    vn = vnatp.tile([128, 130], BF16, tag=f"vn{b}_{kt}", name=f"vn{b}_{kt}")
                    nc.vector.tensor_copy(vn[:, 0:64], ps[:, 0:64])
                    nc.vector.tensor_copy(vn[:, 65:129], ps[:, 64:128])
                    nc.vector.memset(vn[:, 64:65], 1.0)
                    nc.vector.memset(vn[:, 129:130], 1.0)
                    v_nat[b][kt] = vn
                # attention for (b, qc)
                q_sl = slice(b * S + qc * 512, b * S + (qc + 1) * 512)
                nkt = 4 * qc + 4
                o_ps = [
                    psO.tile([65, 512], F32, tag=f"o{h}", name=f"o_ps{h}_{b}_{qc}")
                    for h in (0, 1)
                ]
                for kt in range(nkt):
                    d = 128 * (kt - 4 * qc)  # >=0 on diagonal tiles
                    lo = max(0, d)
                    k_sl = slice(b * S + kt * 128, b * S + (kt + 1) * 128)
                    s_ps = psS.tile([128, 1024], F32, tag="s", name=f"s_{b}_{qc}_{kt}")
                    p_sb = work.tile([128, 1024], BF16, tag="p", name=f"p_{b}_{qc}_{kt}")
                    q_lo = slice(b * S + qc * 512 + lo, b * S + (qc + 1) * 512)
                    for h in (0, 1):
                        hp = slice(64 * h, 64 * h + 64)
                        nc.tensor.matmul(
                            s_ps[:, 512 * h + lo:512 * h + 512],
                            kT[hp, k_sl], qT[hp, q_lo],
                            start=True, stop=True,
                        )
                    nc.scalar.activation(
                        p_sb[:, lo:1024], s_ps[:, lo:1024], EXP, scale=SCALE,
                    )
                    if d >= 0:
                        hi = min(512, d + 128)
                        for h in (0, 1):
                            nc.vector.tensor_mul(
                                p_sb[:, 512 * h + lo:512 * h + hi],
                                p_sb[:, 512 * h + lo:512 * h + hi],
                                mask_sb[:, 0:hi - lo],
                            )
                    for h in (0, 1):
                        nc.tensor.matmul(
                            o_ps[h][:, lo:512],
                            v_nat[b][kt][:, 65 * h:65 * h + 65],
                            p_sb[:, 512 * h + lo:512 * h + 512],
                            start=(kt == 0), stop=(kt == nkt - 1),
                        )
                # normalize (per-head denominator on psum row 64) + stage
                ot = stagep.tile([128, 512], BF16, tag=f"ot{b}_{qc}", name=f"ot{b}_{qc}")
                for h in (0, 1):
                    rc_sb = work.tile([128, 512], F32, tag="recip", name=f"rc{b}_{qc}_{h}")
                    nc.vector.tensor_copy(rc_sb[64:65, :], o_ps[h][64:65, :])
                    nc.vector.tensor_copy(rc_sb[0:1, :], rc_sb[64:65, :])
                    nc.vector.reciprocal_approx_fast(rc_sb[0:1, :], rc_sb[0:1, :])
                    nc.gpsimd.partition_broadcast(
                        rc_sb[0:64, :], rc_sb[0:1, :], channels=64
                    )
                    nc.vector.tensor_mul(
                        ot[64 * h:64 * h + 64, :], o_ps[h][0:64, :], rc_sb[0:64, :]
                    )
                ot_tiles.append((b, qc, ot))

            # ---------- deferred output projection (dense PE tail) ----------
            for b, qc, ot in ot_tiles:
                for rt in range(4):
                    o_sb = outp.tile([128, DM], F32, tag="osb", name=f"osb{b}_{qc}_{rt}")
                    for nc_i in range(2):
                        ps = psmm.tile([128, 512], F32, tag="mm",
                                       name=f"pso{b}_{qc}_{rt}_{nc_i}")
                        nc.tensor.matmul(
                            ps[:], ot[:, rt * 128:(rt + 1) * 128],
                            wo_sb[:, nc_i * 512:(nc_i + 1) * 512],
                            start=True, stop=True,
                        )
                        if nc_i == 0:
                            nc.vector.tensor_copy(
                                o_sb[:, nc_i * 512:(nc_i + 1) * 512], ps[:])
                        else:
                            nc.scalar.copy(
                                o_sb[:, nc_i * 512:(nc_i + 1) * 512], ps[:])
                    nc.sync.dma_start(
                        out_ext[b, qc * 512 + rt * 128: qc * 512 + (rt + 1) * 128, :],
                        o_sb[:],
                    )

    nc.compile()
    return nc


def kernel(x, Wq, bq, Wk, bk, Wv, bv, Wo):
    if "nc" not in _cache:
        _cache["nc"] = _build()
    nc = _cache["nc"]

    bf = ml_dtypes.bfloat16
    xt = np.ascontiguousarray(np.asarray(x, np.float32).reshape(B * S, DM).T).astype(bf)
    wo_f = np.asarray(Wo, np.float32)
    trimask = np.triu(np.ones((128, 128), np.float32))
    ident = np.eye(128, dtype=np.float32)

    in_maps = []
    for c in range(N_CORES):
        sl = slice(c * FPC, (c + 1) * FPC)
        wpk = np.empty((128, 3 * 8 * 128 + DM + 256), np.float32)
        for pr, W in enumerate((Wq, Wk, Wv)):
            Wc = np.asarray(W, np.float32)[:, sl]          # [1024, 128]
            # tile kc: rows [kc*128:(kc+1)*128] -> cols [(pr*8+kc)*128 ...]
            wpk[:, pr * 1024:(pr + 1) * 1024] = (
                Wc.reshape(8, 128, 128).transpose(1, 0, 2).reshape(128, 1024)
            )
        wpk[:, 3072:3072 + DM] = wo_f[sl, :]
        wpk[:, 3072 + DM:3072 + DM + 128] = trimask
        wpk[:, 3072 + DM + 128:] = ident
        bpk = np.stack(
            [np.asarray(b, np.float32)[sl] for b in (bq, bk, bv)], axis=1
        )
        in_maps.append({
            "xt": xt,
            "wpk": np.ascontiguousarray(wpk).astype(bf),
            "bpk": np.ascontiguousarray(bpk),
        })

    trace = bool(int(os.environ.get("ATTN_KERNEL_TRACE", "0")))
    res = run_bass_kernel_spmd(nc, in_maps, core_ids=list(range(N_CORES)), trace=trace)
    if trace:
        print(f"HW exec time: {res.exec_time_ns} ns")
        _cache["exec_time_ns"] = res.exec_time_ns

    out = np.asarray(res.results[0]["out"]).copy()
    for c in range(1, N_CORES):
        out += np.asarray(res.results[c]["out"])
    return out


# revision 25
# speedup vs baseline: 1.0526x; 1.0526x over previous
"""Distributed causal multi-head attention for Trainium2 (8 NeuronCores).

Problem: B=2, S=2048, d_model=1024, 16 heads x 64 dims, causal softmax attention.

Strategy (tensor-parallel over heads + all-to-all for output projection):
  - Each core owns 2 heads (128 of the 1024 QKV features).
  - Host pre-transposes x -> X^T [1024, 4096] and casts inputs to bf16, so all
    on-chip matmuls consume feature-on-partition ("transposed") activations
    directly with no on-chip transposes of x.
  - Per core: Q^T/K^T/V^T = W^T-shard @ X^T (+bias), attention per (batch, head)
    in S^T layout ([k-partitions, q-free]) with exp (no max subtraction; scores
    are O(1) so fp32 exp is safe), causal masking via a single 128x128 upper-
    triangular mask on diagonal tiles, and denominators via an appended ones
    column on V (PE computes the partition-dim sums for free).
  - Both heads of a k-tile share one [128,1024] PSUM tile (adjacent banks) so a
    single ScalarE exp covers them; heads' S^T matmuls pack into the PE array
    via disjoint 64-row groups.
  - Normalization pre-collective (per-head denominators, fast-approx
    reciprocal), then one AllToAll per batch redistributes O^T from head-sharded
    to row-sharded; each core then computes its 2x256 output rows with full Wo.
  - Output f32; host reassembles the full [2, 2048, 1024].
"""
import os
import sys

sys.path.insert(0, "/opt/trn_rl_repo")

import numpy as np
import ml_dtypes

from concourse import bacc, mybir, tile
from concourse.bass_utils import run_bass_kernel_spmd

BF16 = mybir.dt.bfloat16
F32 = mybir.dt.float32

B, S, DM = 2, 2048, 1024
H, DK = 16, 64
N_CORES = 8
FPC = 128           # features per core = 2 heads x 64
RPC = S // N_CORES  # output rows per core per batch = 256
NKT = S // 128      # k-tiles per batch = 16
NQC = S // 512      # q-chunks per batch = 4
SCALE = 1.0 / 8.0   # 1/sqrt(64)

_cache = {}


def _build():
    nc = bacc.Bacc("TRN2", target_bir_lowering=False, debug=False, num_devices=N_CORES)

    xt = nc.dram_tensor("xt", [DM, B * S], BF16, kind="ExternalInput")
    # wpk[p, :]: [wq|wk|wv tiles (3*8*128)] + [wo (1024)] + [mask|ident (256)]
    wpk = nc.dram_tensor("wpk", [128, 3 * 8 * 128 + DM + 256], BF16, kind="ExternalInput")
    bpk = nc.dram_tensor("bpk", [FPC, 3], F32, kind="ExternalInput")
    out_ext = nc.dram_tensor("out", [B, S, DM], BF16, kind="ExternalOutput")

    EXP = mybir.ActivationFunctionType.Exp
    IDENT = mybir.ActivationFunctionType.Identity

    with tile.TileContext(nc) as tc:
        with (
            tc.tile_pool(name="xtp", bufs=1) as xtp,
            tc.tile_pool(name="wts", bufs=1) as wts,
            tc.tile_pool(name="qkv", bufs=1) as qkvp,
            tc.tile_pool(name="vnat", bufs=1) as vnatp,
            tc.tile_pool(name="work", bufs=3) as work,
            tc.tile_pool(name="stage", bufs=2) as stagep,
            tc.tile_pool(name="outp", bufs=2) as outp,
            tc.tile_pool(name="psmm", bufs=2, space="PSUM") as psmm,
            tc.tile_pool(name="psS", bufs=2, space="PSUM") as psS,
            tc.tile_pool(name="psO", bufs=1, space="PSUM") as psO,
        ):
            # ---------- load packed weights/constants (2 DMAs) ----------
            WPK_N = 3 * 8 * 128 + DM + 256
            wpk_sb = wts.tile([128, WPK_N], BF16, tag="wpk", name="wpk_sb")
            nc.sync.dma_start(wpk_sb[:], wpk[:])
            bpk_sb = wts.tile([FPC, 3], F32, tag="bpk", name="bpk_sb")
            nc.sync.dma_start(bpk_sb[:], bpk[:])

            def wslice(pr, kc):
                o = (pr * 8 + kc) * 128
                return wpk_sb[:, o:o + 128]

            wq_sb = [wslice(0, kc) for kc in range(8)]
            wk_sb = [wslice(1, kc) for kc in range(8)]
            wv_sb = [wslice(2, kc) for kc in range(8)]
            wo_sb = wpk_sb[:, 3072:3072 + DM]
            mask_sb = wpk_sb[:, 3072 + DM:3072 + DM + 128]
            ident_sb = wpk_sb[:, 3072 + DM + 128:3072 + DM + 256]
            b_sb = {"q": bpk_sb[:, 0:1], "k": bpk_sb[:, 1:2], "v": bpk_sb[:, 2:3]}

            # xt loaded in column pieces, rc-major, so the first projection
            # chains unblock as soon as the first pieces land
            xt_sb = []
            for kc in range(8):
                t = xtp.tile([128, B * S], BF16, tag=f"xt{kc}", name=f"xt{kc}")
                xt_sb.append(t)
            for rp in range(4):
                for kc in range(8):
                    nc.sync.dma_start(
                        xt_sb[kc][:, rp * 1024:(rp + 1) * 1024],
                        xt[kc * 128:(kc + 1) * 128, rp * 1024:(rp + 1) * 1024],
                    )

            # ---------- phases 1-3 interleaved ----------
            # row-chunk rc feeds q-chunk (b, qi): projections for rc, then V
            # transposes for that q range, then attention for (b, qi). This
            # overlaps ScalarE exp work with TensorE projection matmuls.
            proj_sb = {}
            for name in ("q", "k", "v"):
                proj_sb[name] = qkvp.tile(
                    [128, B * S], BF16, tag=f"{name}T", name=f"{name}T"
                )
            qT, kT, vT = proj_sb["q"], proj_sb["k"], proj_sb["v"]
            w_by_name = {"q": wq_sb, "k": wk_sb, "v": wv_sb}
            v_nat = [[None] * NKT for _ in range(B)]
            ot_tiles = []

            for rc in range(8):
                b, qc = (0, rc) if rc < 4 else (1, rc - 4)
                # projections for this row chunk
                for name in ("q", "k", "v"):
                    ps = psmm.tile([128, 512], F32, tag="mm", name=f"ps_{name}{rc}")
                    for kc in range(8):
                        nc.tensor.matmul(
                            ps[:], w_by_name[name][kc],
                            xt_sb[kc][:, rc * 512:(rc + 1) * 512],
                            start=(kc == 0), stop=(kc == 7),
                        )
                    nc.scalar.activation(
                        proj_sb[name][:, rc * 512:(rc + 1) * 512], ps[:], IDENT,
                        bias=b_sb[name],
                    )
                # V natural (+ones cols) for this q range
                for kt in range(4 * qc, 4 * qc + 4):
                    ps = psmm.tile([128, 128], BF16, tag="mm", name=f"pst{b}_{kt}")
                    nc.tensor.transpose(
                        ps[:], vT[:, b * S + kt * 128: b * S + (kt + 1) * 128],
                        ident_sb,
                    )
                    vn = vnatp.tile([128, 130], BF16, tag=f"vn{b}_{kt}", name=f"vn{b}_{kt}")
                    nc.vector.tensor_copy(vn[:, 0:64], ps[:, 0:64])
                    nc.vector.tensor_copy(vn[:, 65:129], ps[:, 64:128])
                    nc.vector.memset(vn[:, 64:65], 1.0)
                    nc.vector.memset(vn[:, 129:130], 1.0)
                    v_nat[b][kt] = vn
                # attention for (b, qc)
                q_sl = slice(b * S + qc * 512, b * S + (qc + 1) * 512)
                nkt = 4 * qc + 4
                o_ps = [
                    psO.tile([65, 512], F32, tag=f"o{h}", name=f"o_ps{h}_{b}_{qc}")
                    for h in (0, 1)
                ]
                for kt in range(nkt):
                    d = 128 * (kt - 4 * qc)  # >=0 on diagonal tiles
                    lo = max(0, d)
                    k_sl = slice(b * S + kt * 128, b * S + (kt + 1) * 128)
                    s_ps = psS.tile([128, 1024], F32, tag="s", name=f"s_{b}_{qc}_{kt}")
                    p_sb = work.tile([128, 1024], BF16, tag="p", name=f"p_{b}_{qc}_{kt}")
                    q_lo = slice(b * S + qc * 512 + lo, b * S + (qc + 1) * 512)
                    for h in (0, 1):
                        hp = slice(64 * h, 64 * h + 64)
                        nc.tensor.matmul(
                            s_ps[:, 512 * h + lo:512 * h + 512],
                            kT[hp, k_sl], qT[hp, q_lo],
                            start=True, stop=True,
                        )
                    nc.scalar.activation(
                        p_sb[:, lo:1024], s_ps[:, lo:1024], EXP, scale=SCALE,
                    )
                    if d >= 0:
                        hi = min(512, d + 128)
                        for h in (0, 1):
                            nc.vector.tensor_mul(
                                p_sb[:, 512 * h + lo:512 * h + hi],
                                p_sb[:, 512 * h + lo:512 * h + hi],
                                mask_sb[:, 0:hi - lo],
                            )
                    for h in (0, 1):
                        nc.tensor.matmul(
                            o_ps[h][:, lo:512],
                            v_nat[b][kt][:, 65 * h:65 * h + 65],
                            p_sb[:, 512 * h + lo:512 * h + 512],
                            start=(kt == 0), stop=(kt == nkt - 1),
                        )
                # normalize (per-head denominator on psum row 64) + stage
                ot = stagep.tile([128, 512], BF16, tag=f"ot{b}_{qc}", name=f"ot{b}_{qc}")
                for h in (0, 1):
                    rc_sb = work.tile([128, 512], F32, tag="recip", name=f"rc{b}_{qc}_{h}")
                    nc.vector.tensor_copy(rc_sb[64:65, :], o_ps[h][64:65, :])
                    nc.vector.tensor_copy(rc_sb[0:1, :], rc_sb[64:65, :])
                    nc.vector.reciprocal_approx_fast(rc_sb[0:1, :], rc_sb[0:1, :])
                    nc.gpsimd.partition_broadcast(
                        rc_sb[0:64, :], rc_sb[0:1, :], channels=64
                    )
                    nc.vector.tensor_mul(
                        ot[64 * h:64 * h + 64, :], o_ps[h][0:64, :], rc_sb[0:64, :]
                    )
                ot_tiles.append((b, qc, ot))

            # ---------- deferred output projection (dense PE tail) ----------
            for b, qc, ot in ot_tiles:
                for rt in range(4):
                    o_sb = outp.tile([128, DM], BF16, tag="osb", name=f"osb{b}_{qc}_{rt}")
                    for nc_i in range(2):
                        ps = psmm.tile([128, 512], F32, tag="mm",
                                       name=f"pso{b}_{qc}_{rt}_{nc_i}")
                        nc.tensor.matmul(
                            ps[:], ot[:, rt * 128:(rt + 1) * 128],
                            wo_sb[:, nc_i * 512:(nc_i + 1) * 512],
                            start=True, stop=True,
                        )
                        if nc_i == 0:
                            nc.vector.tensor_copy(
                                o_sb[:, nc_i * 512:(nc_i + 1) * 512], ps[:])
                        else:
                            nc.scalar.copy(
                                o_sb[:, nc_i * 512:(nc_i + 1) * 512], ps[:])
                    nc.sync.dma_start(
                        out_ext[b, qc * 512 + rt * 128: qc * 512 + (rt + 1) * 128, :],
                        o_sb[:],
                    )

    nc.compile()
    return nc


def kernel(x, Wq, bq, Wk, bk, Wv, bv, Wo):
    if "nc" not in _cache:
        _cache["nc"] = _build()
    nc = _cache["nc"]

    bf = ml_dtypes.bfloat16
    xt = np.ascontiguousarray(np.asarray(x, np.float32).reshape(B * S, DM).T).astype(bf)
    wo_f = np.asarray(Wo, np.float32)
    trimask = np.triu(np.ones((128, 128), np.float32))
    ident = np.eye(128, dtype=np.float32)

    in_maps = []
    for c in range(N_CORES):
        sl = slice(c * FPC, (c + 1) * FPC)
        wpk = np.empty((128, 3 * 8 * 128 + DM + 256), np.float32)
        for pr, W in enumerate((Wq, Wk, Wv)):
            Wc = np.asarray(W, np.float32)[:, sl]          # [1024, 128]
            # tile kc: rows [kc*128:(kc+1)*128] -> cols [(pr*8+kc)*128 ...]
            wpk[:, pr * 1024:(pr + 1) * 1024] = (
                Wc.reshape(8, 128, 128).transpose(1, 0, 2).reshape(128, 1024)
            )
        wpk[:, 3072:3072 + DM] = wo_f[sl, :]
        wpk[:, 3072 + DM:3072 + DM + 128] = trimask
        wpk[:, 3072 + DM + 128:] = ident
        bpk = np.stack(
            [np.asarray(b, np.float32)[sl] for b in (bq, bk, bv)], axis=1
        )
        in_maps.append({
            "xt": xt,
            "wpk": np.ascontiguousarray(wpk).astype(bf),
            "bpk": np.ascontiguousarray(bpk),
        })

    trace = bool(int(os.environ.get("ATTN_KERNEL_TRACE", "0")))
    res = run_bass_kernel_spmd(nc, in_maps, core_ids=list(range(N_CORES)), trace=trace)
    if trace:
        print(f"HW exec time: {res.exec_time_ns} ns")
        _cache["exec_time_ns"] = res.exec_time_ns

    out = np.asarray(res.results[0]["out"]).astype(np.float32)
    for c in range(1, N_CORES):
        out += np.asarray(res.results[c]["out"]).astype(np.float32)
    return out


# revision 26
# speedup vs baseline: 1.0618x; 1.0087x over previous
"""Distributed causal multi-head attention for Trainium2 (8 NeuronCores).

Problem: B=2, S=2048, d_model=1024, 16 heads x 64 dims, causal softmax attention.

Strategy (tensor-parallel over heads + all-to-all for output projection):
  - Each core owns 2 heads (128 of the 1024 QKV features).
  - Host pre-transposes x -> X^T [1024, 4096] and casts inputs to bf16, so all
    on-chip matmuls consume feature-on-partition ("transposed") activations
    directly with no on-chip transposes of x.
  - Per core: Q^T/K^T/V^T = W^T-shard @ X^T (+bias), attention per (batch, head)
    in S^T layout ([k-partitions, q-free]) with exp (no max subtraction; scores
    are O(1) so fp32 exp is safe), causal masking via a single 128x128 upper-
    triangular mask on diagonal tiles, and denominators via an appended ones
    column on V (PE computes the partition-dim sums for free).
  - Both heads of a k-tile share one [128,1024] PSUM tile (adjacent banks) so a
    single ScalarE exp covers them; heads' S^T matmuls pack into the PE array
    via disjoint 64-row groups.
  - Normalization pre-collective (per-head denominators, fast-approx
    reciprocal), then one AllToAll per batch redistributes O^T from head-sharded
    to row-sharded; each core then computes its 2x256 output rows with full Wo.
  - Output f32; host reassembles the full [2, 2048, 1024].
"""
import os
import sys

sys.path.insert(0, "/opt/trn_rl_repo")

import numpy as np
import ml_dtypes

from concourse import bacc, mybir, tile
from concourse.bass_utils import run_bass_kernel_spmd

BF16 = mybir.dt.bfloat16
F32 = mybir.dt.float32

B, S, DM = 2, 2048, 1024
H, DK = 16, 64
N_CORES = 8
FPC = 128           # features per core = 2 heads x 64
RPC = S // N_CORES  # output rows per core per batch = 256
NKT = S // 128      # k-tiles per batch = 16
NQC = S // 512      # q-chunks per batch = 4
SCALE = 1.0 / 8.0   # 1/sqrt(64)

_cache = {}


def _build():
    nc = bacc.Bacc("TRN2", target_bir_lowering=False, debug=False, num_devices=N_CORES)

    xt = nc.dram_tensor("xt", [DM, B * S], BF16, kind="ExternalInput")
    # wpk[p, :]: [wq|wk|wv tiles (3*8*128)] + [wo (1024)] + [mask|ident (256)]
    wpk = nc.dram_tensor("wpk", [128, 3 * 8 * 128 + DM + 256], BF16, kind="ExternalInput")
    bpk = nc.dram_tensor("bpk", [FPC, 3], F32, kind="ExternalInput")
    out_ext = nc.dram_tensor("out", [B, S, DM], BF16, kind="ExternalOutput")

    EXP = mybir.ActivationFunctionType.Exp
    IDENT = mybir.ActivationFunctionType.Identity

    with tile.TileContext(nc) as tc:
        with (
            tc.tile_pool(name="xtp", bufs=1) as xtp,
            tc.tile_pool(name="wts", bufs=1) as wts,
            tc.tile_pool(name="qkv", bufs=1) as qkvp,
            tc.tile_pool(name="vnat", bufs=1) as vnatp,
            tc.tile_pool(name="work", bufs=3) as work,
            tc.tile_pool(name="stage", bufs=2) as stagep,
            tc.tile_pool(name="outp", bufs=2) as outp,
            tc.tile_pool(name="psmm", bufs=2, space="PSUM") as psmm,
            tc.tile_pool(name="psS", bufs=2, space="PSUM") as psS,
            tc.tile_pool(name="psO", bufs=1, space="PSUM") as psO,
        ):
            # ---------- load packed weights/constants (2 DMAs) ----------
            WPK_N = 3 * 8 * 128 + DM + 256
            wpk_sb = wts.tile([128, WPK_N], BF16, tag="wpk", name="wpk_sb")
            nc.sync.dma_start(wpk_sb[:], wpk[:])
            bpk_sb = wts.tile([FPC, 3], F32, tag="bpk", name="bpk_sb")
            nc.sync.dma_start(bpk_sb[:], bpk[:])

            def wslice(pr, kc):
                o = (pr * 8 + kc) * 128
                return wpk_sb[:, o:o + 128]

            wq_sb = [wslice(0, kc) for kc in range(8)]
            wk_sb = [wslice(1, kc) for kc in range(8)]
            wv_sb = [wslice(2, kc) for kc in range(8)]
            wo_sb = wpk_sb[:, 3072:3072 + DM]
            mask_sb = wpk_sb[:, 3072 + DM:3072 + DM + 128]
            ident_sb = wpk_sb[:, 3072 + DM + 128:3072 + DM + 256]
            b_sb = {"q": bpk_sb[:, 0:1], "k": bpk_sb[:, 1:2], "v": bpk_sb[:, 2:3]}

            # xt loaded in column pieces, rc-major, so the first projection
            # chains unblock as soon as the first pieces land
            xt_sb = []
            for kc in range(8):
                t = xtp.tile([128, B * S], BF16, tag=f"xt{kc}", name=f"xt{kc}")
                xt_sb.append(t)
            for rp in range(4):
                for kc in range(8):
                    nc.sync.dma_start(
                        xt_sb[kc][:, rp * 1024:(rp + 1) * 1024],
                        xt[kc * 128:(kc + 1) * 128, rp * 1024:(rp + 1) * 1024],
                    )

            # ---------- phases 1-3 interleaved ----------
            # row-chunk rc feeds q-chunk (b, qi): projections for rc, then V
            # transposes for that q range, then attention for (b, qi). This
            # overlaps ScalarE exp work with TensorE projection matmuls.
            proj_sb = {}
            for name in ("q", "k", "v"):
                proj_sb[name] = qkvp.tile(
                    [128, B * S], BF16, tag=f"{name}T", name=f"{name}T"
                )
            qT, kT, vT = proj_sb["q"], proj_sb["k"], proj_sb["v"]
            w_by_name = {"q": wq_sb, "k": wk_sb, "v": wv_sb}
            v_nat = [[None] * NKT for _ in range(B)]
            ot_tiles = []

            for rc in range(8):
                b, qc = (0, rc) if rc < 4 else (1, rc - 4)
                # projections for this row chunk
                for name in ("q", "k", "v"):
                    ps = psmm.tile([128, 512], F32, tag="mm", name=f"ps_{name}{rc}")
                    for kc in range(8):
                        nc.tensor.matmul(
                            ps[:], w_by_name[name][kc],
                            xt_sb[kc][:, rc * 512:(rc + 1) * 512],
                            start=(kc == 0), stop=(kc == 7),
                        )
                    nc.scalar.activation(
                        proj_sb[name][:, rc * 512:(rc + 1) * 512], ps[:], IDENT,
                        bias=b_sb[name],
                    )
                # V natural (+ones cols) for this q range
                for kt in range(4 * qc, 4 * qc + 4):
                    ps = psmm.tile([128, 128], BF16, tag="mm", name=f"pst{b}_{kt}")
                    nc.tensor.transpose(
                        ps[:], vT[:, b * S + kt * 128: b * S + (kt + 1) * 128],
                        ident_sb,
                    )
                    vn = vnatp.tile([128, 130], BF16, tag=f"vn{b}_{kt}", name=f"vn{b}_{kt}")
                    nc.vector.tensor_copy(vn[:, 0:64], ps[:, 0:64])
                    nc.vector.tensor_copy(vn[:, 65:129], ps[:, 64:128])
                    nc.vector.memset(vn[:, 64:65], 1.0)
                    nc.vector.memset(vn[:, 129:130], 1.0)
                    v_nat[b][kt] = vn
                # attention for (b, qc)
                q_sl = slice(b * S + qc * 512, b * S + (qc + 1) * 512)
                nkt = 4 * qc + 4
                o_ps = [
                    psO.tile([65, 512], F32, tag=f"o{h}", name=f"o_ps{h}_{b}_{qc}")
                    for h in (0, 1)
                ]
                def emit_s(kt):
                    d = 128 * (kt - 4 * qc)
                    lo = max(0, d)
                    k_sl = slice(b * S + kt * 128, b * S + (kt + 1) * 128)
                    s_ps = psS.tile([128, 1024], F32, tag="s", name=f"s_{b}_{qc}_{kt}")
                    q_lo = slice(b * S + qc * 512 + lo, b * S + (qc + 1) * 512)
                    for h in (0, 1):
                        hp = slice(64 * h, 64 * h + 64)
                        nc.tensor.matmul(
                            s_ps[:, 512 * h + lo:512 * h + 512],
                            kT[hp, k_sl], qT[hp, q_lo],
                            start=True, stop=True,
                        )
                    return s_ps, lo, d

                s_cur = emit_s(0)
                for kt in range(nkt):
                    s_ps, lo, d = s_cur
                    s_nxt = emit_s(kt + 1) if kt + 1 < nkt else None
                    p_sb = work.tile([128, 1024], BF16, tag="p", name=f"p_{b}_{qc}_{kt}")
                    nc.scalar.activation(
                        p_sb[:, lo:1024], s_ps[:, lo:1024], EXP, scale=SCALE,
                    )
                    if d >= 0:
                        hi = min(512, d + 128)
                        for h in (0, 1):
                            nc.vector.tensor_mul(
                                p_sb[:, 512 * h + lo:512 * h + hi],
                                p_sb[:, 512 * h + lo:512 * h + hi],
                                mask_sb[:, 0:hi - lo],
                            )
                    for h in (0, 1):
                        nc.tensor.matmul(
                            o_ps[h][:, lo:512],
                            v_nat[b][kt][:, 65 * h:65 * h + 65],
                            p_sb[:, 512 * h + lo:512 * h + 512],
                            start=(kt == 0), stop=(kt == nkt - 1),
                        )
                    s_cur = s_nxt
                # normalize (per-head denominator on psum row 64) + stage
                ot = stagep.tile([128, 512], BF16, tag=f"ot{b}_{qc}", name=f"ot{b}_{qc}")
                for h in (0, 1):
                    rc_sb = work.tile([128, 512], F32, tag="recip", name=f"rc{b}_{qc}_{h}")
                    nc.vector.tensor_copy(rc_sb[64:65, :], o_ps[h][64:65, :])
                    nc.vector.tensor_copy(rc_sb[0:1, :], rc_sb[64:65, :])
                    nc.vector.reciprocal_approx_fast(rc_sb[0:1, :], rc_sb[0:1, :])
                    nc.gpsimd.partition_broadcast(
                        rc_sb[0:64, :], rc_sb[0:1, :], channels=64
                    )
                    nc.vector.tensor_mul(
                        ot[64 * h:64 * h + 64, :], o_ps[h][0:64, :], rc_sb[0:64, :]
                    )
                ot_tiles.append((b, qc, ot))

            # ---------- deferred output projection (dense PE tail) ----------
            for b, qc, ot in ot_tiles:
                for rt in range(4):
                    o_sb = outp.tile([128, DM], BF16, tag="osb", name=f"osb{b}_{qc}_{rt}")
                    for nc_i in range(2):
                        ps = psmm.tile([128, 512], F32, tag="mm",
                                       name=f"pso{b}_{qc}_{rt}_{nc_i}")
                        nc.tensor.matmul(
                            ps[:], ot[:, rt * 128:(rt + 1) * 128],
                            wo_sb[:, nc_i * 512:(nc_i + 1) * 512],
                            start=True, stop=True,
                        )
                        if nc_i == 0:
                            nc.vector.tensor_copy(
                                o_sb[:, nc_i * 512:(nc_i + 1) * 512], ps[:])
                        else:
                            nc.scalar.copy(
                                o_sb[:, nc_i * 512:(nc_i + 1) * 512], ps[:])
                    nc.sync.dma_start(
                        out_ext[b, qc * 512 + rt * 128: qc * 512 + (rt + 1) * 128, :],
                        o_sb[:],
                    )

    nc.compile()
    return nc


def kernel(x, Wq, bq, Wk, bk, Wv, bv, Wo):
    if "nc" not in _cache:
        _cache["nc"] = _build()
    nc = _cache["nc"]

    bf = ml_dtypes.bfloat16
    xt = np.ascontiguousarray(np.asarray(x, np.float32).reshape(B * S, DM).T).astype(bf)
    wo_f = np.asarray(Wo, np.float32)
    trimask = np.triu(np.ones((128, 128), np.float32))
    ident = np.eye(128, dtype=np.float32)

    in_maps = []
    for c in range(N_CORES):
        sl = slice(c * FPC, (c + 1) * FPC)
        wpk = np.empty((128, 3 * 8 * 128 + DM + 256), np.float32)
        for pr, W in enumerate((Wq, Wk, Wv)):
            Wc = np.asarray(W, np.float32)[:, sl]          # [1024, 128]
            # tile kc: rows [kc*128:(kc+1)*128] -> cols [(pr*8+kc)*128 ...]
            wpk[:, pr * 1024:(pr + 1) * 1024] = (
                Wc.reshape(8, 128, 128).transpose(1, 0, 2).reshape(128, 1024)
            )
        wpk[:, 3072:3072 + DM] = wo_f[sl, :]
        wpk[:, 3072 + DM:3072 + DM + 128] = trimask
        wpk[:, 3072 + DM + 128:] = ident
        bpk = np.stack(
            [np.asarray(b, np.float32)[sl] for b in (bq, bk, bv)], axis=1
        )
        in_maps.append({
            "xt": xt,
            "wpk": np.ascontiguousarray(wpk).astype(bf),
            "bpk": np.ascontiguousarray(bpk),
        })

    trace = bool(int(os.environ.get("ATTN_KERNEL_TRACE", "0")))
    res = run_bass_kernel_spmd(nc, in_maps, core_ids=list(range(N_CORES)), trace=trace)
    if trace:
        print(f"HW exec time: {res.exec_time_ns} ns")
        _cache["exec_time_ns"] = res.exec_time_ns

    out = np.asarray(res.results[0]["out"]).astype(np.float32)
    for c in range(1, N_CORES):
        out += np.asarray(res.results[c]["out"]).astype(np.float32)
    return out
